# revision 4
# baseline (speedup 1.0000x reference)
"""Trainium2 Bass kernel for GNN message passing (edge-conditioned conv + GRU).

Math (per batch b, E=128 edges, N=64 atoms, D=64, BD=32):
  A[e]      = sum_d bf[e,d] * bt3[d]          (per-edge DxD matrix)
  msg[e]    = A[e]^T-contracted src_atom[e]   = sum_{d,l} bf[e,d]*srcA[e,l]*bt3[d,l,:]
  agg[n]    = sum_{e: tgt[e]==n} msg[e]
  out       = GRU(x=agg, h=atom)              (Keras reset_after GRU, one step)

Kernel formulation (per core, 32 batches, data-parallel over batch):
  - gather srcA^T via one-hot matmul; lhsT = [atom|atom] doubles rows for free
  - KR^T chunk c (K-rows = (j,l), d=2c+j): srcA2 * bfE_c, where bfE_c is the
    bf-row expansion produced by a tiny selection matmul on the PE
  - msg^T = sum_c W2_c^T @ KR_c  (K=2048 contraction, W2 = reshaped bond_transform)
  - transpose msg^T per batch on PE, scatter-add via one-hot matmul -> agg^T
  - GRU on natural [2*N, D] layout; biases folded in as K=1 ones-row matmuls
"""

import sys

sys.path.insert(0, "/opt/trn_rl_repo")

import numpy as np

B, N, E, D, BD = 256, 64, 128, 64, 32
NCORES = 8
BPC = B // NCORES          # batches per core = 32
BLK = 4                    # batches per column-block (4*128 = 512 cols)
NBLK = BPC // BLK          # 8 blocks per core
NCHUNK = (BD * D) // 128   # 16 K-chunks of 128 (2 d-values x 64 l)


def _build_nc():
    import concourse.bacc as bacc
    import concourse.mybir as mybir
    from concourse.tile import TileContext

    f32 = mybir.dt.float32
    AF = mybir.ActivationFunctionType
    OP = mybir.AluOpType

    nc = bacc.Bacc(None, target_bir_lowering=False)

    # per-core inputs
    atom_d = nc.declare_dram_parameter("atom", [BPC * N, D], f32, isOutput=False)
    atomT2_d = nc.declare_dram_parameter("atomT2", [BPC // 2, D, 2 * N], f32, isOutput=False)
    bfT_d = nc.declare_dram_parameter("bfT", [BD, BPC * E], f32, isOutput=False)
    srcf_d = nc.declare_dram_parameter("srcf", [BPC, E], f32, isOutput=False)
    tgtf_d = nc.declare_dram_parameter("tgtf", [BPC, E], f32, isOutput=False)
    # replicated constants
    w2_d = nc.declare_dram_parameter("w2", [128, NCHUNK * D], f32, isOutput=False)
    sel_d = nc.declare_dram_parameter("sel", [BD, NCHUNK * 128], f32, isOutput=False)
    ident_d = nc.declare_dram_parameter("ident", [N, N], f32, isOutput=False)
    iotac_d = nc.declare_dram_parameter("iotac", [N, 1], f32, isOutput=False)
    iotar_d = nc.declare_dram_parameter("iotar", [E, N], f32, isOutput=False)
    gruk_d = nc.declare_dram_parameter("gruk", [D, 3 * D], f32, isOutput=False)
    grur_d = nc.declare_dram_parameter("grur", [D, 3 * D], f32, isOutput=False)
    ones_d = nc.declare_dram_parameter("ones", [1, 2 * N], f32, isOutput=False)
    bias_d = nc.declare_dram_parameter("bias", [1, 4 * D], f32, isOutput=False)
    out_d = nc.declare_dram_parameter("out", [BPC * N, D], f32, isOutput=True)

    with TileContext(nc) as tc:
        with (
            tc.tile_pool(name="consts", bufs=1) as cpool,
            tc.tile_pool(name="sb_in", bufs=2) as inpool,
            tc.tile_pool(name="sb_oh", bufs=2) as ohpool,
            tc.tile_pool(name="sb_blk", bufs=2) as blkpool,
            tc.tile_pool(name="sb_kr", bufs=3) as krpool,
            tc.tile_pool(name="sb_msg", bufs=2) as msgpool,
            tc.tile_pool(name="sb_gru", bufs=2) as grupool,
            tc.tile_pool(name="ps_bfe", bufs=2, space="PSUM") as ps_bfe,
            tc.tile_pool(name="ps_msgT", bufs=1, space="PSUM") as ps_msgT,
            tc.tile_pool(name="ps_sa", bufs=2, space="PSUM") as ps_sa,
            tc.tile_pool(name="ps_misc", bufs=2, space="PSUM") as ps_misc,
            tc.tile_pool(name="ps_gru", bufs=1, space="PSUM") as ps_gru,
        ):
            # ---- constants ----
            w2_c = cpool.tile([128, NCHUNK * D], f32)
            nc.sync.dma_start(out=w2_c[:], in_=w2_d[:])
            sel_c = cpool.tile([BD, NCHUNK * 128], f32)
            nc.sync.dma_start(out=sel_c[:], in_=sel_d[:])
            ident_c = cpool.tile([N, N], f32)
            nc.sync.dma_start(out=ident_c[:], in_=ident_d[:])
            iotac_c = cpool.tile([N, 1], f32)
            nc.sync.dma_start(out=iotac_c[:], in_=iotac_d[:])
            iotar_c = cpool.tile([E, N], f32)
            nc.sync.dma_start(out=iotar_c[:], in_=iotar_d[:])
            gruk_c = cpool.tile([D, 3 * D], f32)
            nc.sync.dma_start(out=gruk_c[:], in_=gruk_d[:])
            grur_c = cpool.tile([D, 3 * D], f32)
            nc.sync.dma_start(out=grur_c[:], in_=grur_d[:])
            ones_c = cpool.tile([1, 2 * N], f32)
            nc.sync.dma_start(out=ones_c[:], in_=ones_d[:])
            bias_c = cpool.tile([1, 4 * D], f32)
            nc.sync.dma_start(out=bias_c[:], in_=bias_d[:])

            for blk in range(NBLK):
                # bf^T slice for this block's 512 edge-columns
                bfTb = blkpool.tile([BD, BLK * E], f32, tag="bfTb")
                nc.sync.dma_start(
                    out=bfTb[:], in_=bfT_d[:, blk * BLK * E : (blk + 1) * BLK * E]
                )
                srcA2 = blkpool.tile([128, BLK * E], f32, tag="srcA2")
                tgts = []
                for i in range(BLK):
                    b = blk * BLK + i  # local batch index
                    # --- one-hot builds ---
                    srcrow = inpool.tile([1, E], f32, tag="srcrow")
                    nc.sync.dma_start(out=srcrow[:], in_=srcf_d[b : b + 1, :])
                    srcB = ohpool.tile([N, E], f32, tag="srcB")
                    nc.gpsimd.partition_broadcast(srcB[:], srcrow[:])
                    s_srcT = ohpool.tile([N, E], f32, tag="s_srcT")
                    nc.vector.tensor_tensor(
                        out=s_srcT[:],
                        in0=iotac_c[:].to_broadcast([N, E]),
                        in1=srcB[:],
                        op=OP.is_equal,
                    )
                    tgtcol = inpool.tile([E, 1], f32, tag="tgtcol")
                    nc.sync.dma_start(out=tgtcol[:], in_=tgtf_d[b, :, None])
                    t_tgt = ohpool.tile([E, N], f32, tag="t_tgt")
                    nc.vector.tensor_tensor(
                        out=t_tgt[:],
                        in0=tgtcol[:].to_broadcast([E, N]),
                        in1=iotar_c[:],
                        op=OP.is_equal,
                    )
                    tgts.append(t_tgt)
                    # --- gather: srcA2[:, i*128:(i+1)*128] = doubled srcAtoms^T ---
                    atom2 = inpool.tile([N, 2 * D], f32, tag="atom2")
                    nc.sync.dma_start(
                        out=atom2[:, 0:D], in_=atom_d[b * N : (b + 1) * N, :]
                    )
                    nc.sync.dma_start(
                        out=atom2[:, D : 2 * D], in_=atom_d[b * N : (b + 1) * N, :]
                    )
                    ps = ps_sa.tile([128, E], f32, tag="ps_sa")
                    nc.tensor.matmul(
                        out=ps[:], lhsT=atom2[:], rhs=s_srcT[:], start=True, stop=True
                    )
                    nc.scalar.activation(
                        srcA2[:, i * E : (i + 1) * E], ps[:], AF.Copy
                    )

                # --- main contraction: msg^T[m, 512cols] over K=2048 in 16 chunks ---
                mT = ps_msgT.tile([D, BLK * E], f32, tag="msgT")
                for c in range(NCHUNK):
                    bfe = ps_bfe.tile([128, BLK * E], f32, tag="bfe")
                    nc.tensor.matmul(
                        out=bfe[:],
                        lhsT=sel_c[:, c * 128 : (c + 1) * 128],
                        rhs=bfTb[:],
                        start=True,
                        stop=True,
                    )
                    kr = krpool.tile([128, BLK * E], f32, tag="kr")
                    nc.vector.tensor_tensor(
                        out=kr[:], in0=srcA2[:], in1=bfe[:], op=OP.mult
                    )
                    nc.tensor.matmul(
                        out=mT[:],
                        lhsT=w2_c[:, c * D : (c + 1) * D],
                        rhs=kr[:],
                        start=(c == 0),
                        stop=(c == NCHUNK - 1),
                    )
                msgTs = msgpool.tile([D, BLK * E], f32, tag="msgTs")
                nc.scalar.activation(msgTs[:], mT[:], AF.Copy)

                # --- per batch: transpose + scatter ---
                for i in range(BLK):
                    b = blk * BLK + i
                    if i % 2 == 0:
                        xT2 = grupool.tile([D, 2 * N], f32, tag="xT2")
                    psm = ps_misc.tile([E, D], f32, tag="ps_misc")
                    nc.tensor.transpose(
                        out=psm[:],
                        in_=msgTs[:, i * E : (i + 1) * E],
                        identity=ident_c[:],
                    )
                    msgs = msgpool.tile([E, D], f32, tag="msgs")
                    nc.scalar.activation(msgs[:], psm[:], AF.Copy)
                    psa = ps_misc.tile([D, N], f32, tag="ps_misc")
                    nc.tensor.matmul(
                        out=psa[:], lhsT=msgs[:], rhs=tgts[i][:], start=True, stop=True
                    )
                    nc.scalar.activation(
                        xT2[:, (i % 2) * N : (i % 2 + 1) * N], psa[:], AF.Copy
                    )

                    # --- GRU for the completed 2-batch pair ---
                    if i % 2 == 1:
                        k2 = b // 2  # pair index within core
                        hT = inpool.tile([D, 2 * N], f32, tag="hT")
                        nc.sync.dma_start(out=hT[:], in_=atomT2_d[k2])
                        hnat = inpool.tile([2 * N, D], f32, tag="hnat")
                        nc.sync.dma_start(
                            out=hnat[:], in_=atom_d[k2 * 2 * N : (k2 + 1) * 2 * N, :]
                        )
                        pg = ps_gru.tile([2 * N, 4 * D], f32, tag="ps_gru")
                        # z|r gates: x@Wk[:,:2D] + h@Wr[:,:2D] + (b0+b1)[:2D]
                        nc.tensor.matmul(
                            out=pg[:, 0 : 2 * D], lhsT=xT2[:], rhs=gruk_c[:, 0 : 2 * D],
                            start=True, stop=False,
                        )
                        nc.tensor.matmul(
                            out=pg[:, 0 : 2 * D], lhsT=hT[:], rhs=grur_c[:, 0 : 2 * D],
                            start=False, stop=False,
                        )
                        nc.tensor.matmul(
                            out=pg[:, 0 : 2 * D], lhsT=ones_c[:], rhs=bias_c[:, 0 : 2 * D],
                            start=False, stop=True,
                        )
                        # xh = x@Wk[:,2D:] + b0[2D:]
                        nc.tensor.matmul(
                            out=pg[:, 2 * D : 3 * D], lhsT=xT2[:],
                            rhs=gruk_c[:, 2 * D : 3 * D], start=True, stop=False,
                        )
                        nc.tensor.matmul(
                            out=pg[:, 2 * D : 3 * D], lhsT=ones_c[:],
                            rhs=bias_c[:, 2 * D : 3 * D], start=False, stop=True,
                        )
                        # hh = h@Wr[:,2D:] + b1[2D:]
                        nc.tensor.matmul(
                            out=pg[:, 3 * D : 4 * D], lhsT=hT[:],
                            rhs=grur_c[:, 2 * D : 3 * D], start=True, stop=False,
                        )
                        nc.tensor.matmul(
                            out=pg[:, 3 * D : 4 * D], lhsT=ones_c[:],
                            rhs=bias_c[:, 3 * D : 4 * D], start=False, stop=True,
                        )
                        zr = grupool.tile([2 * N, 2 * D], f32, tag="zr")
                        nc.scalar.activation(zr[:], pg[:, 0 : 2 * D], AF.Sigmoid)
                        m1 = grupool.tile([2 * N, D], f32, tag="m1")
                        nc.vector.tensor_tensor(
                            out=m1[:], in0=zr[:, D : 2 * D], in1=pg[:, 3 * D : 4 * D],
                            op=OP.mult,
                        )
                        s2 = grupool.tile([2 * N, D], f32, tag="s2")
                        nc.vector.tensor_tensor(
                            out=s2[:], in0=m1[:], in1=pg[:, 2 * D : 3 * D], op=OP.add
                        )
                        hc = grupool.tile([2 * N, D], f32, tag="hc")
                        nc.scalar.activation(hc[:], s2[:], AF.Tanh)
                        d1 = grupool.tile([2 * N, D], f32, tag="d1")
                        nc.vector.tensor_tensor(
                            out=d1[:], in0=hnat[:], in1=hc[:], op=OP.subtract
                        )
                        m2 = grupool.tile([2 * N, D], f32, tag="m2")
                        nc.vector.tensor_tensor(
                            out=m2[:], in0=zr[:, 0:D], in1=d1[:], op=OP.mult
                        )
                        hnew = grupool.tile([2 * N, D], f32, tag="hnew")
                        nc.vector.tensor_tensor(
                            out=hnew[:], in0=hc[:], in1=m2[:], op=OP.add
                        )
                        nc.sync.dma_start(
                            out=out_d[k2 * 2 * N : (k2 + 1) * 2 * N, :], in_=hnew[:]
                        )

    nc.finalize()
    return nc


_NC_CACHE = {}


def _get_nc():
    if "nc" not in _NC_CACHE:
        _NC_CACHE["nc"] = _build_nc()
    return _NC_CACHE["nc"]


def kernel(
    atom_features,
    bond_features,
    connectivity,
    bond_transform,
    gru_kernel,
    gru_rec_kernel,
    gru_bias,
):
    from concourse.bass_utils import run_bass_kernel_spmd

    atom_features = np.asarray(atom_features, dtype=np.float32)
    bond_features = np.asarray(bond_features, dtype=np.float32)
    connectivity = np.asarray(connectivity)
    bond_transform = np.asarray(bond_transform, dtype=np.float32)
    gru_kernel = np.asarray(gru_kernel, dtype=np.float32)
    gru_rec_kernel = np.asarray(gru_rec_kernel, dtype=np.float32)
    gru_bias = np.asarray(gru_bias, dtype=np.float32)

    # ---- host-side layout prep (no FLOPs of the actual math) ----
    # W2[(d,l), m] = bt3[d, l, m]; chunk c rows (j,l) with d = 2c+j are rows
    # [128c, 128c+128) of the d-major flattening -> pack chunks along columns.
    w2 = (
        bond_transform.reshape(NCHUNK, 128, D).transpose(1, 0, 2).reshape(128, NCHUNK * D)
    )
    sel = np.zeros((BD, NCHUNK * 128), dtype=np.float32)
    for c in range(NCHUNK):
        p = np.arange(128)
        sel[2 * c + p // 64, c * 128 + p] = 1.0
    ident = np.eye(N, dtype=np.float32)
    iotac = np.arange(N, dtype=np.float32).reshape(N, 1)
    iotar = np.broadcast_to(np.arange(N, dtype=np.float32), (E, N)).copy()
    ones = np.ones((1, 2 * N), dtype=np.float32)
    b0, b1 = gru_bias[0], gru_bias[1]
    bias = np.concatenate([(b0 + b1)[: 2 * D], b0[2 * D :], b1[2 * D :]]).reshape(1, 4 * D)

    atomT2 = (
        atom_features.transpose(0, 2, 1)
        .reshape(B // 2, 2, D, N)
        .transpose(0, 2, 1, 3)
        .reshape(B // 2, D, 2 * N)
    )
    bfT_all = bond_features.reshape(B * E, BD).T.copy()  # [BD, B*E]
    srcf = connectivity[:, :, 0].astype(np.float32)
    tgtf = connectivity[:, :, 1].astype(np.float32)
    atom_flat = atom_features.reshape(B * N, D)

    shared = {
        "w2": np.ascontiguousarray(w2),
        "sel": sel,
        "ident": ident,
        "iotac": iotac,
        "iotar": iotar,
        "gruk": np.ascontiguousarray(gru_kernel),
        "grur": np.ascontiguousarray(gru_rec_kernel),
        "ones": ones,
        "bias": np.ascontiguousarray(bias),
    }
    in_maps = []
    for k in range(NCORES):
        bs = slice(k * BPC, (k + 1) * BPC)
        in_maps.append(
            {
                "atom": np.ascontiguousarray(atom_flat[k * BPC * N : (k + 1) * BPC * N]),
                "atomT2": np.ascontiguousarray(atomT2[k * BPC // 2 : (k + 1) * BPC // 2]),
                "bfT": np.ascontiguousarray(bfT_all[:, k * BPC * E : (k + 1) * BPC * E]),
                "srcf": np.ascontiguousarray(srcf[bs]),
                "tgtf": np.ascontiguousarray(tgtf[bs]),
                **shared,
            }
        )

    nc = _get_nc()
    import os

    kw = {}
    if os.environ.get("KTRACE_DIR"):
        kw["tmpdir"] = os.environ["KTRACE_DIR"]
    res = run_bass_kernel_spmd(nc, in_maps, list(range(NCORES)), **kw)
    out = np.concatenate([res.results[k]["out"] for k in range(NCORES)], axis=0)
    _NC_CACHE["last_results"] = res
    return out.reshape(B, N, D)


# revision 7
# speedup vs baseline: 1.9941x; 1.9941x over previous
"""Trainium2 Bass kernel for GNN message passing (edge-conditioned conv + GRU).

Math (per batch b, E=128 edges, N=64 atoms, D=64, BD=32):
  A[e]      = sum_d bf[e,d] * bt3[d]          (per-edge DxD matrix)
  msg[e]    = A[e]^T-contracted src_atom[e]   = sum_{d,l} bf[e,d]*srcA[e,l]*bt3[d,l,:]
  agg[n]    = sum_{e: tgt[e]==n} msg[e]
  out       = GRU(x=agg, h=atom)              (Keras reset_after GRU, one step)

Kernel formulation (per core, 32 batches, data-parallel over batch):
  - gather srcA^T via one-hot matmul; lhsT = [atom|atom] doubles rows for free
  - KR^T chunk c (K-rows = (j,l), d=2c+j): srcA2 * bfE_c, where bfE_c is the
    bf-row expansion produced by a tiny selection matmul on the PE
  - msg^T = sum_c W2_c^T @ KR_c  (K=2048 contraction, W2 = reshaped bond_transform)
  - transpose msg^T per batch on PE, scatter-add via one-hot matmul -> agg^T
  - GRU on natural [2*N, D] layout; biases folded in as K=1 ones-row matmuls
"""

import sys

sys.path.insert(0, "/opt/trn_rl_repo")

import numpy as np

try:
    import ml_dtypes

    BF16 = ml_dtypes.bfloat16
except ImportError:  # pragma: no cover
    BF16 = None

B, N, E, D, BD = 256, 64, 128, 64, 32
NCORES = 8
BPC = B // NCORES          # batches per core = 32
BLK = 4                    # batches per column-block (4*128 = 512 cols)
NBLK = BPC // BLK          # 8 blocks per core
NCHUNK = (BD * D) // 128   # 16 K-chunks of 128 (2 d-values x 64 l)


def _build_nc():
    import concourse.bacc as bacc
    import concourse.mybir as mybir
    from concourse.tile import TileContext

    f32 = mybir.dt.float32
    bf16 = mybir.dt.bfloat16
    AF = mybir.ActivationFunctionType
    OP = mybir.AluOpType

    nc = bacc.Bacc(None, target_bir_lowering=False)

    # per-core inputs
    atom_d = nc.declare_dram_parameter("atom", [BPC * N, D], f32, isOutput=False)
    atomb_d = nc.declare_dram_parameter("atomb", [BPC * N, D], bf16, isOutput=False)
    atomT2_d = nc.declare_dram_parameter("atomT2", [BPC // 2, D, 2 * N], bf16, isOutput=False)
    bfT_d = nc.declare_dram_parameter("bfT", [BD, BPC * E], bf16, isOutput=False)
    srcf_d = nc.declare_dram_parameter("srcf", [BPC, E], f32, isOutput=False)
    tgtf_d = nc.declare_dram_parameter("tgtf", [E, BPC], f32, isOutput=False)
    # replicated constants
    w2_d = nc.declare_dram_parameter("w2", [128, NCHUNK * D], bf16, isOutput=False)
    sel_d = nc.declare_dram_parameter("sel", [BD, NCHUNK * 128], bf16, isOutput=False)
    ident_d = nc.declare_dram_parameter("ident", [N, N], bf16, isOutput=False)
    iotac_d = nc.declare_dram_parameter("iotac", [N, 1], f32, isOutput=False)
    iotar_d = nc.declare_dram_parameter("iotar", [E, N], f32, isOutput=False)
    gruk_d = nc.declare_dram_parameter("gruk", [D, 3 * D], bf16, isOutput=False)
    grur_d = nc.declare_dram_parameter("grur", [D, 3 * D], bf16, isOutput=False)
    ones_d = nc.declare_dram_parameter("ones", [1, 2 * N], bf16, isOutput=False)
    bias_d = nc.declare_dram_parameter("bias", [1, 4 * D], bf16, isOutput=False)
    out_d = nc.declare_dram_parameter("out", [BPC * N, D], f32, isOutput=True)

    with TileContext(nc) as tc:
        with (
            tc.tile_pool(name="consts", bufs=1) as cpool,
            tc.tile_pool(name="sb_in", bufs=2) as inpool,
            tc.tile_pool(name="sb_oh", bufs=2) as ohpool,
            tc.tile_pool(name="sb_blk", bufs=2) as blkpool,
            tc.tile_pool(name="sb_kr", bufs=3) as krpool,
            tc.tile_pool(name="sb_msg", bufs=2) as msgpool,
            tc.tile_pool(name="sb_gru", bufs=2) as grupool,
            tc.tile_pool(name="ps_bfe", bufs=2, space="PSUM") as ps_bfe,
            tc.tile_pool(name="ps_msgT", bufs=1, space="PSUM") as ps_msgT,
            tc.tile_pool(name="ps_sa", bufs=2, space="PSUM") as ps_sa,
            tc.tile_pool(name="ps_misc", bufs=2, space="PSUM") as ps_misc,
            tc.tile_pool(name="ps_gru", bufs=1, space="PSUM") as ps_gru,
        ):
            # ---- constants ----
            w2_c = cpool.tile([128, NCHUNK * D], bf16)
            nc.sync.dma_start(out=w2_c[:], in_=w2_d[:])
            sel_c = cpool.tile([BD, NCHUNK * 128], bf16)
            nc.sync.dma_start(out=sel_c[:], in_=sel_d[:])
            ident_c = cpool.tile([N, N], bf16)
            nc.sync.dma_start(out=ident_c[:], in_=ident_d[:])
            iotac_c = cpool.tile([N, 1], f32)
            nc.sync.dma_start(out=iotac_c[:], in_=iotac_d[:])
            iotar_c = cpool.tile([E, N], f32)
            nc.sync.dma_start(out=iotar_c[:], in_=iotar_d[:])
            gruk_c = cpool.tile([D, 3 * D], bf16)
            nc.sync.dma_start(out=gruk_c[:], in_=gruk_d[:])
            grur_c = cpool.tile([D, 3 * D], bf16)
            nc.sync.dma_start(out=grur_c[:], in_=grur_d[:])
            ones_c = cpool.tile([1, 2 * N], bf16)
            nc.sync.dma_start(out=ones_c[:], in_=ones_d[:])
            bias_c = cpool.tile([1, 4 * D], bf16)
            nc.sync.dma_start(out=bias_c[:], in_=bias_d[:])

            for blk in range(NBLK):
                # bf^T slice for this block's 512 edge-columns
                bfTb = blkpool.tile([BD, BLK * E], bf16, tag="bfTb")
                nc.sync.dma_start(
                    out=bfTb[:], in_=bfT_d[:, blk * BLK * E : (blk + 1) * BLK * E]
                )
                srcA2 = blkpool.tile([128, BLK * E], f32, tag="srcA2")
                tgtblk = blkpool.tile([E, BLK], f32, tag="tgtblk")
                nc.sync.dma_start(
                    out=tgtblk[:], in_=tgtf_d[:, blk * BLK : (blk + 1) * BLK]
                )
                tgts = []
                for i in range(BLK):
                    b = blk * BLK + i  # local batch index
                    # --- one-hot builds ---
                    srcrow = inpool.tile([1, E], f32, tag="srcrow")
                    nc.sync.dma_start(out=srcrow[:], in_=srcf_d[b : b + 1, :])
                    srcB = ohpool.tile([N, E], f32, tag="srcB")
                    nc.gpsimd.partition_broadcast(srcB[:], srcrow[:])
                    s_srcT = ohpool.tile([N, E], bf16, tag="s_srcT")
                    nc.vector.tensor_tensor(
                        out=s_srcT[:],
                        in0=iotac_c[:].to_broadcast([N, E]),
                        in1=srcB[:],
                        op=OP.is_equal,
                    )
                    t_tgt = ohpool.tile([E, N], bf16, tag="t_tgt")
                    nc.vector.tensor_tensor(
                        out=t_tgt[:],
                        in0=tgtblk[:, i : i + 1].to_broadcast([E, N]),
                        in1=iotar_c[:],
                        op=OP.is_equal,
                    )
                    tgts.append(t_tgt)
                    # --- gather: srcA2[:, i*128:(i+1)*128] = doubled srcAtoms^T ---
                    atom2 = inpool.tile([N, 2 * D], bf16, tag="atom2")
                    nc.gpsimd.dma_start(
                        out=atom2[:, 0:D], in_=atomb_d[b * N : (b + 1) * N, :]
                    )
                    nc.gpsimd.dma_start(
                        out=atom2[:, D : 2 * D], in_=atomb_d[b * N : (b + 1) * N, :]
                    )
                    ps = ps_sa.tile([128, E], f32, tag="ps_sa")
                    nc.tensor.matmul(
                        out=ps[:], lhsT=atom2[:], rhs=s_srcT[:], start=True, stop=True
                    )
                    nc.scalar.activation(
                        srcA2[:, i * E : (i + 1) * E], ps[:], AF.Copy
                    )

                # --- main contraction: msg^T[m, 512cols] over K=2048 in 16 chunks ---
                mT = ps_msgT.tile([D, BLK * E], f32, tag="msgT")
                for c in range(NCHUNK):
                    bfe = ps_bfe.tile([128, BLK * E], f32, tag="bfe")
                    nc.tensor.matmul(
                        out=bfe[:],
                        lhsT=sel_c[:, c * 128 : (c + 1) * 128],
                        rhs=bfTb[:],
                        start=True,
                        stop=True,
                    )
                    kr = krpool.tile([128, BLK * E], bf16, tag="kr")
                    nc.vector.tensor_tensor(
                        out=kr[:], in0=srcA2[:], in1=bfe[:], op=OP.mult
                    )
                    nc.tensor.matmul(
                        out=mT[:],
                        lhsT=w2_c[:, c * D : (c + 1) * D],
                        rhs=kr[:],
                        start=(c == 0),
                        stop=(c == NCHUNK - 1),
                    )
                msgTs = msgpool.tile([D, BLK * E], bf16, tag="msgTs")
                nc.scalar.activation(msgTs[:], mT[:], AF.Copy)

                # --- per batch: transpose + scatter ---
                for i in range(BLK):
                    b = blk * BLK + i
                    if i % 2 == 0:
                        xT2 = grupool.tile([D, 2 * N], bf16, tag="xT2")
                    psm = ps_misc.tile([E, D], bf16, tag="ps_misc")
                    nc.tensor.transpose(
                        out=psm[:],
                        in_=msgTs[:, i * E : (i + 1) * E],
                        identity=ident_c[:],
                    )
                    msgs = msgpool.tile([E, D], bf16, tag="msgs")
                    nc.scalar.activation(msgs[:], psm[:], AF.Copy)
                    psa = ps_misc.tile([D, N], f32, tag="ps_misc")
                    nc.tensor.matmul(
                        out=psa[:], lhsT=msgs[:], rhs=tgts[i][:], start=True, stop=True
                    )
                    nc.scalar.activation(
                        xT2[:, (i % 2) * N : (i % 2 + 1) * N], psa[:], AF.Copy
                    )

                    # --- GRU for the completed 2-batch pair ---
                    if i % 2 == 1:
                        k2 = b // 2  # pair index within core
                        hT = inpool.tile([D, 2 * N], bf16, tag="hT")
                        nc.gpsimd.dma_start(out=hT[:], in_=atomT2_d[k2])
                        hnat = inpool.tile([2 * N, D], f32, tag="hnat")
                        nc.sync.dma_start(
                            out=hnat[:], in_=atom_d[k2 * 2 * N : (k2 + 1) * 2 * N, :]
                        )
                        pg = ps_gru.tile([2 * N, 4 * D], f32, tag="ps_gru")
                        # z|r gates: x@Wk[:,:2D] + h@Wr[:,:2D] + (b0+b1)[:2D]
                        nc.tensor.matmul(
                            out=pg[:, 0 : 2 * D], lhsT=xT2[:], rhs=gruk_c[:, 0 : 2 * D],
                            start=True, stop=False,
                        )
                        nc.tensor.matmul(
                            out=pg[:, 0 : 2 * D], lhsT=hT[:], rhs=grur_c[:, 0 : 2 * D],
                            start=False, stop=False,
                        )
                        nc.tensor.matmul(
                            out=pg[:, 0 : 2 * D], lhsT=ones_c[:], rhs=bias_c[:, 0 : 2 * D],
                            start=False, stop=True,
                        )
                        # xh = x@Wk[:,2D:] + b0[2D:]
                        nc.tensor.matmul(
                            out=pg[:, 2 * D : 3 * D], lhsT=xT2[:],
                            rhs=gruk_c[:, 2 * D : 3 * D], start=True, stop=False,
                        )
                        nc.tensor.matmul(
                            out=pg[:, 2 * D : 3 * D], lhsT=ones_c[:],
                            rhs=bias_c[:, 2 * D : 3 * D], start=False, stop=True,
                        )
                        # hh = h@Wr[:,2D:] + b1[2D:]
                        nc.tensor.matmul(
                            out=pg[:, 3 * D : 4 * D], lhsT=hT[:],
                            rhs=grur_c[:, 2 * D : 3 * D], start=True, stop=False,
                        )
                        nc.tensor.matmul(
                            out=pg[:, 3 * D : 4 * D], lhsT=ones_c[:],
                            rhs=bias_c[:, 3 * D : 4 * D], start=False, stop=True,
                        )
                        zr = grupool.tile([2 * N, 2 * D], f32, tag="zr")
                        nc.scalar.activation(zr[:], pg[:, 0 : 2 * D], AF.Sigmoid)
                        m1 = grupool.tile([2 * N, D], f32, tag="m1")
                        nc.vector.tensor_tensor(
                            out=m1[:], in0=zr[:, D : 2 * D], in1=pg[:, 3 * D : 4 * D],
                            op=OP.mult,
                        )
                        s2 = grupool.tile([2 * N, D], f32, tag="s2")
                        nc.vector.tensor_tensor(
                            out=s2[:], in0=m1[:], in1=pg[:, 2 * D : 3 * D], op=OP.add
                        )
                        hc = grupool.tile([2 * N, D], f32, tag="hc")
                        nc.scalar.activation(hc[:], s2[:], AF.Tanh)
                        d1 = grupool.tile([2 * N, D], f32, tag="d1")
                        nc.vector.tensor_tensor(
                            out=d1[:], in0=hnat[:], in1=hc[:], op=OP.subtract
                        )
                        m2 = grupool.tile([2 * N, D], f32, tag="m2")
                        nc.vector.tensor_tensor(
                            out=m2[:], in0=zr[:, 0:D], in1=d1[:], op=OP.mult
                        )
                        hnew = grupool.tile([2 * N, D], f32, tag="hnew")
                        nc.vector.tensor_tensor(
                            out=hnew[:], in0=hc[:], in1=m2[:], op=OP.add
                        )
                        nc.sync.dma_start(
                            out=out_d[k2 * 2 * N : (k2 + 1) * 2 * N, :], in_=hnew[:]
                        )

    nc.finalize()
    return nc


_NC_CACHE = {}


def _get_nc():
    if "nc" not in _NC_CACHE:
        _NC_CACHE["nc"] = _build_nc()
    return _NC_CACHE["nc"]


def kernel(
    atom_features,
    bond_features,
    connectivity,
    bond_transform,
    gru_kernel,
    gru_rec_kernel,
    gru_bias,
):
    from concourse.bass_utils import run_bass_kernel_spmd

    atom_features = np.asarray(atom_features, dtype=np.float32)
    bond_features = np.asarray(bond_features, dtype=np.float32)
    connectivity = np.asarray(connectivity)
    bond_transform = np.asarray(bond_transform, dtype=np.float32)
    gru_kernel = np.asarray(gru_kernel, dtype=np.float32)
    gru_rec_kernel = np.asarray(gru_rec_kernel, dtype=np.float32)
    gru_bias = np.asarray(gru_bias, dtype=np.float32)

    # ---- host-side layout prep (no FLOPs of the actual math) ----
    # W2[(d,l), m] = bt3[d, l, m]; chunk c rows (j,l) with d = 2c+j are rows
    # [128c, 128c+128) of the d-major flattening -> pack chunks along columns.
    w2 = (
        bond_transform.reshape(NCHUNK, 128, D).transpose(1, 0, 2).reshape(128, NCHUNK * D)
    )
    sel = np.zeros((BD, NCHUNK * 128), dtype=np.float32)
    for c in range(NCHUNK):
        p = np.arange(128)
        sel[2 * c + p // 64, c * 128 + p] = 1.0
    ident = np.eye(N, dtype=np.float32)
    iotac = np.arange(N, dtype=np.float32).reshape(N, 1)
    iotar = np.broadcast_to(np.arange(N, dtype=np.float32), (E, N)).copy()
    ones = np.ones((1, 2 * N), dtype=np.float32)
    b0, b1 = gru_bias[0], gru_bias[1]
    bias = np.concatenate([(b0 + b1)[: 2 * D], b0[2 * D :], b1[2 * D :]]).reshape(1, 4 * D)

    atomT2 = (
        atom_features.transpose(0, 2, 1)
        .reshape(B // 2, 2, D, N)
        .transpose(0, 2, 1, 3)
        .reshape(B // 2, D, 2 * N)
    )
    bfT_all = bond_features.reshape(B * E, BD).T.copy()  # [BD, B*E]
    srcf = connectivity[:, :, 0].astype(np.float32)
    tgtf = connectivity[:, :, 1].astype(np.float32)
    atom_flat = atom_features.reshape(B * N, D)

    shared = {
        "w2": np.ascontiguousarray(w2).astype(BF16),
        "sel": sel.astype(BF16),
        "ident": ident.astype(BF16),
        "iotac": iotac,
        "iotar": iotar,
        "gruk": np.ascontiguousarray(gru_kernel).astype(BF16),
        "grur": np.ascontiguousarray(gru_rec_kernel).astype(BF16),
        "ones": ones.astype(BF16),
        "bias": np.ascontiguousarray(bias).astype(BF16),
    }
    in_maps = []
    for k in range(NCORES):
        bs = slice(k * BPC, (k + 1) * BPC)
        in_maps.append(
            {
                "atom": np.ascontiguousarray(atom_flat[k * BPC * N : (k + 1) * BPC * N]),
                "atomb": np.ascontiguousarray(
                    atom_flat[k * BPC * N : (k + 1) * BPC * N]
                ).astype(BF16),
                "atomT2": np.ascontiguousarray(
                    atomT2[k * BPC // 2 : (k + 1) * BPC // 2]
                ).astype(BF16),
                "bfT": np.ascontiguousarray(
                    bfT_all[:, k * BPC * E : (k + 1) * BPC * E]
                ).astype(BF16),
                "srcf": np.ascontiguousarray(srcf[bs]),
                "tgtf": np.ascontiguousarray(tgtf[bs].T),
                **shared,
            }
        )

    nc = _get_nc()
    import os

    kw = {}
    if os.environ.get("KTRACE_DIR"):
        kw["tmpdir"] = os.environ["KTRACE_DIR"]
    res = run_bass_kernel_spmd(nc, in_maps, list(range(NCORES)), **kw)
    out = np.concatenate([res.results[k]["out"] for k in range(NCORES)], axis=0)
    _NC_CACHE["last_results"] = res
    return out.reshape(B, N, D)


# revision 10
# speedup vs baseline: 2.1675x; 1.0869x over previous
"""Trainium2 Bass kernel for GNN message passing (edge-conditioned conv + GRU).

Math (per batch b, E=128 edges, N=64 atoms, D=64, BD=32):
  A[e]      = sum_d bf[e,d] * bt3[d]          (per-edge DxD matrix)
  msg[e]    = sum_{d,l} bf[e,d]*srcA[e,l]*bt3[d,l,:]
  agg[n]    = sum_{e: tgt[e]==n} msg[e]
  out       = GRU(x=agg, h=atom)              (Keras reset_after GRU, one step)

Kernel formulation (per core, 32 batches, data-parallel over batch):
  - all inputs SBUF-resident (one DMA each); bf16 on every matmul operand,
    fp32 PSUM accumulation
  - gather srcA^T via one-hot matmul; lhsT free-broadcast doubles rows
  - KR^T chunk c (K-rows = (j,l), d=2c+j) = srcA2 * bfE_c; bfE_c produced by
    K=32 selection matmuls, 4 chunks packed concurrently via row-group tiling
  - msg^T = sum_c W2_c^T @ KR_c  (K=2048, W2 = reshaped bond_transform)
  - per-batch PE transpose, scatter-add via one-hot matmul -> agg^T
  - GRU on natural [2*N, D] layout; biases folded in as K=1 ones-row matmuls
"""

import sys

sys.path.insert(0, "/opt/trn_rl_repo")

import numpy as np

try:
    import ml_dtypes

    BF16 = ml_dtypes.bfloat16
except ImportError:  # pragma: no cover
    BF16 = None

B, N, E, D, BD = 256, 64, 128, 64, 32
NCORES = 8
BPC = B // NCORES          # batches per core = 32
BLK = 4                    # batches per column-block (4*128 = 512 cols)
NBLK = BPC // BLK          # 8 blocks per core
NCHUNK = (BD * D) // 128   # 16 K-chunks of 128 (2 d-values x 64 l)
NPAIR = BPC // 2


def _build_nc():
    import concourse.bacc as bacc
    import concourse.mybir as mybir
    from concourse.tile import TileContext

    f32 = mybir.dt.float32
    bf16 = mybir.dt.bfloat16
    AF = mybir.ActivationFunctionType
    OP = mybir.AluOpType

    nc = bacc.Bacc(None, target_bir_lowering=False)

    # per-core inputs (resident layouts)
    atomg_d = nc.declare_dram_parameter("atomg", [N, BPC * 2 * D], bf16, isOutput=False)
    atomT_d = nc.declare_dram_parameter("atomT", [D, BPC * N], bf16, isOutput=False)
    hnat_d = nc.declare_dram_parameter("hnat", [2 * N, NPAIR * D], f32, isOutput=False)
    bfT4_d = nc.declare_dram_parameter("bfT4", [128, BPC * E], bf16, isOutput=False)
    srcf_d = nc.declare_dram_parameter("srcf", [1, BPC * E], f32, isOutput=False)
    tgtf_d = nc.declare_dram_parameter("tgtf", [E, BPC], f32, isOutput=False)
    # replicated constants
    w2_d = nc.declare_dram_parameter("w2", [128, NCHUNK * D], bf16, isOutput=False)
    selp_d = nc.declare_dram_parameter(
        "selp", [128, (NCHUNK // 4) * 128], bf16, isOutput=False
    )
    ident_d = nc.declare_dram_parameter("ident", [N, N], bf16, isOutput=False)
    iotac_d = nc.declare_dram_parameter("iotac", [N, 1], f32, isOutput=False)
    iotar4_d = nc.declare_dram_parameter("iotar4", [E, BLK * N], f32, isOutput=False)
    gruk_d = nc.declare_dram_parameter("gruk", [D, 3 * D], bf16, isOutput=False)
    grur_d = nc.declare_dram_parameter("grur", [D, 3 * D], bf16, isOutput=False)
    ones_d = nc.declare_dram_parameter("ones", [1, 2 * N], bf16, isOutput=False)
    biasx_d = nc.declare_dram_parameter("biasx", [1, 4 * D], bf16, isOutput=False)
    out_d = nc.declare_dram_parameter("out", [2 * N, NPAIR * D], f32, isOutput=True)

    with TileContext(nc) as tc:
        with (
            tc.tile_pool(name="consts", bufs=1) as cpool,
            tc.tile_pool(name="sb_oh", bufs=2) as ohpool,
            tc.tile_pool(name="sb_blk", bufs=2) as blkpool,
            tc.tile_pool(name="sb_kr", bufs=4) as krpool,
            tc.tile_pool(name="sb_msg", bufs=2) as msgpool,
            tc.tile_pool(name="sb_gru", bufs=2) as grupool,
            tc.tile_pool(name="sb_out", bufs=1) as outpool,
            tc.tile_pool(name="ps_bfe", bufs=4, space="PSUM") as ps_bfe,
            tc.tile_pool(name="ps_msgT", bufs=1, space="PSUM") as ps_msgT,
            tc.tile_pool(name="ps_sa", bufs=1, space="PSUM") as ps_sa,
            tc.tile_pool(name="ps_misc", bufs=1, space="PSUM") as ps_misc,
            tc.tile_pool(name="ps_gru", bufs=1, space="PSUM") as ps_gru,
        ):
            # ---- resident inputs / constants (one DMA each, spread queues) ----
            atomg_c = cpool.tile([N, BPC * 2 * D], bf16)
            nc.sync.dma_start(out=atomg_c[:], in_=atomg_d[:])
            atomT_c = cpool.tile([D, BPC * N], bf16)
            nc.gpsimd.dma_start(out=atomT_c[:], in_=atomT_d[:])
            hnat_c = cpool.tile([2 * N, NPAIR * D], f32)
            nc.scalar.dma_start(out=hnat_c[:], in_=hnat_d[:])
            bfT4_c = cpool.tile([128, BPC * E], bf16)
            nc.scalar.dma_start(out=bfT4_c[:], in_=bfT4_d[:])
            srcf_c = cpool.tile([1, BPC * E], f32)
            nc.sync.dma_start(out=srcf_c[:], in_=srcf_d[:])
            tgtf_c = cpool.tile([E, BPC], f32)
            nc.gpsimd.dma_start(out=tgtf_c[:], in_=tgtf_d[:])
            w2_c = cpool.tile([128, NCHUNK * D], bf16)
            nc.sync.dma_start(out=w2_c[:], in_=w2_d[:])
            selp_c = cpool.tile([128, (NCHUNK // 4) * 128], bf16)
            nc.gpsimd.dma_start(out=selp_c[:], in_=selp_d[:])
            ident_c = cpool.tile([N, N], bf16)
            nc.scalar.dma_start(out=ident_c[:], in_=ident_d[:])
            iotac_c = cpool.tile([N, 1], f32)
            nc.sync.dma_start(out=iotac_c[:], in_=iotac_d[:])
            iotar4_c = cpool.tile([E, BLK * N], f32)
            nc.sync.dma_start(out=iotar4_c[:], in_=iotar4_d[:])
            gruk_c = cpool.tile([D, 3 * D], bf16)
            nc.gpsimd.dma_start(out=gruk_c[:], in_=gruk_d[:])
            grur_c = cpool.tile([D, 3 * D], bf16)
            nc.scalar.dma_start(out=grur_c[:], in_=grur_d[:])
            ones_c = cpool.tile([1, 2 * N], bf16)
            nc.gpsimd.dma_start(out=ones_c[:], in_=ones_d[:])
            biasx_c = cpool.tile([1, 4 * D], bf16)
            nc.sync.dma_start(out=biasx_c[:], in_=biasx_d[:])
            outbig = outpool.tile([2 * N, NPAIR * D], f32)

            for blk in range(NBLK):
                c0, c1 = blk * BLK * E, (blk + 1) * BLK * E
                # --- batched one-hot builds for the 4 batches of this block ---
                srcB = ohpool.tile([N, BLK * E], f32, tag="srcB")
                nc.gpsimd.partition_broadcast(srcB[:], srcf_c[:, c0:c1])
                s_srcT4 = ohpool.tile([N, BLK * E], bf16, tag="s_srcT4")
                nc.vector.tensor_tensor(
                    out=s_srcT4[:],
                    in0=iotac_c[:].to_broadcast([N, BLK * E]),
                    in1=srcB[:],
                    op=OP.is_equal,
                )
                t_tgt4 = ohpool.tile([E, BLK * N], bf16, tag="t_tgt4")
                nc.vector.tensor_tensor(
                    out=t_tgt4[:].rearrange("e (i n) -> e i n", i=BLK),
                    in0=tgtf_c[:, blk * BLK : (blk + 1) * BLK]
                    .unsqueeze(2)
                    .to_broadcast([E, BLK, N]),
                    in1=iotar4_c[:].rearrange("e (i n) -> e i n", i=BLK),
                    op=OP.is_equal,
                )
                # --- gather: 4 matmuls into one PSUM bank, one copy out ---
                ps = ps_sa.tile([128, BLK * E], f32, tag="ps_sa")
                for i in range(BLK):
                    b = blk * BLK + i
                    nc.tensor.matmul(
                        out=ps[:, i * E : (i + 1) * E],
                        lhsT=atomg_c[:, b * 2 * D : (b + 1) * 2 * D],
                        rhs=s_srcT4[:, i * E : (i + 1) * E],
                        start=True,
                        stop=True,
                    )
                srcA2 = blkpool.tile([128, BLK * E], f32, tag="srcA2")
                nc.scalar.activation(srcA2[:], ps[:], AF.Copy)

                # --- main contraction: bfE 4-packed, 16 chunks accumulate ---
                mT = ps_msgT.tile([D, BLK * E], f32, tag="msgT")
                for r in range(NCHUNK // 4):
                    bfes = []
                    for j in range(4):
                        bfe = ps_bfe.tile([128, BLK * E], f32, tag="bfe")
                        nc.tensor.matmul(
                            out=bfe[:],
                            lhsT=selp_c[
                                32 * j : 32 * (j + 1), r * 128 : (r + 1) * 128
                            ],
                            rhs=bfT4_c[32 * j : 32 * (j + 1), c0:c1],
                            start=True,
                            stop=True,
                            tile_position=(32 * j, 0),
                        )
                        bfes.append(bfe)
                    for j in range(4):
                        c = r * 4 + j
                        kr = krpool.tile([128, BLK * E], bf16, tag="kr")
                        nc.vector.tensor_tensor(
                            out=kr[:], in0=srcA2[:], in1=bfes[j][:], op=OP.mult
                        )
                        nc.tensor.matmul(
                            out=mT[:],
                            lhsT=w2_c[:, c * D : (c + 1) * D],
                            rhs=kr[:],
                            start=(c == 0),
                            stop=(c == NCHUNK - 1),
                        )
                msgTs = msgpool.tile([D, BLK * E], bf16, tag="msgTs")
                nc.scalar.activation(msgTs[:], mT[:], AF.Copy)

                # --- per batch: transpose + scatter; GRU per completed pair ---
                for i in range(BLK):
                    b = blk * BLK + i
                    if i % 2 == 0:
                        xT2 = grupool.tile([D, 2 * N], bf16, tag="xT2")
                    psm = ps_misc.tile([E, D], bf16, tag="ps_misc")
                    nc.tensor.transpose(
                        out=psm[:],
                        in_=msgTs[:, i * E : (i + 1) * E],
                        identity=ident_c[:],
                    )
                    msgs = msgpool.tile([E, D], bf16, tag="msgs")
                    nc.scalar.activation(msgs[:], psm[:], AF.Copy)
                    psa = ps_misc.tile([D, N], f32, tag="ps_misc")
                    nc.tensor.matmul(
                        out=psa[:],
                        lhsT=msgs[:],
                        rhs=t_tgt4[:, i * N : (i + 1) * N],
                        start=True,
                        stop=True,
                    )
                    nc.scalar.activation(
                        xT2[:, (i % 2) * N : (i % 2 + 1) * N], psa[:], AF.Copy
                    )

                    if i % 2 == 1:
                        k2 = b // 2
                        pg = ps_gru.tile([2 * N, 4 * D], f32, tag="ps_gru")
                        nc.tensor.matmul(
                            out=pg[:, 0 : 3 * D], lhsT=xT2[:], rhs=gruk_c[:],
                            start=True, stop=False, skip_group_check=True,
                        )
                        nc.tensor.matmul(
                            out=pg[:, 0 : 2 * D],
                            lhsT=atomT_c[:, k2 * 2 * N : (k2 + 1) * 2 * N],
                            rhs=grur_c[:, 0 : 2 * D],
                            start=False, stop=False, skip_group_check=True,
                        )
                        nc.tensor.matmul(
                            out=pg[:, 0 : 3 * D], lhsT=ones_c[:],
                            rhs=biasx_c[:, 0 : 3 * D],
                            start=False, stop=True, skip_group_check=True,
                        )
                        nc.tensor.matmul(
                            out=pg[:, 3 * D : 4 * D],
                            lhsT=atomT_c[:, k2 * 2 * N : (k2 + 1) * 2 * N],
                            rhs=grur_c[:, 2 * D : 3 * D],
                            start=True, stop=False, skip_group_check=True,
                        )
                        nc.tensor.matmul(
                            out=pg[:, 3 * D : 4 * D], lhsT=ones_c[:],
                            rhs=biasx_c[:, 3 * D : 4 * D],
                            start=False, stop=True, skip_group_check=True,
                        )
                        zr = grupool.tile([2 * N, 2 * D], f32, tag="zr")
                        nc.scalar.activation(zr[:], pg[:, 0 : 2 * D], AF.Sigmoid)
                        m1 = grupool.tile([2 * N, D], f32, tag="m1")
                        nc.vector.tensor_tensor(
                            out=m1[:], in0=zr[:, D : 2 * D],
                            in1=pg[:, 3 * D : 4 * D], op=OP.mult,
                        )
                        s2 = grupool.tile([2 * N, D], f32, tag="s2")
                        nc.vector.tensor_tensor(
                            out=s2[:], in0=m1[:], in1=pg[:, 2 * D : 3 * D], op=OP.add
                        )
                        hc = grupool.tile([2 * N, D], f32, tag="hc")
                        nc.scalar.activation(hc[:], s2[:], AF.Tanh)
                        d1 = grupool.tile([2 * N, D], f32, tag="d1")
                        nc.vector.tensor_tensor(
                            out=d1[:],
                            in0=hnat_c[:, k2 * D : (k2 + 1) * D],
                            in1=hc[:],
                            op=OP.subtract,
                        )
                        m2 = grupool.tile([2 * N, D], f32, tag="m2")
                        nc.vector.tensor_tensor(
                            out=m2[:], in0=zr[:, 0:D], in1=d1[:], op=OP.mult
                        )
                        nc.vector.tensor_tensor(
                            out=outbig[:, k2 * D : (k2 + 1) * D],
                            in0=hc[:],
                            in1=m2[:],
                            op=OP.add,
                        )
            nc.sync.dma_start(out=out_d[:], in_=outbig[:])

    nc.finalize()
    return nc


_NC_CACHE = {}


def _get_nc():
    if "nc" not in _NC_CACHE:
        _NC_CACHE["nc"] = _build_nc()
    return _NC_CACHE["nc"]


def kernel(
    atom_features,
    bond_features,
    connectivity,
    bond_transform,
    gru_kernel,
    gru_rec_kernel,
    gru_bias,
):
    from concourse.bass_utils import run_bass_kernel_spmd

    atom_features = np.asarray(atom_features, dtype=np.float32)
    bond_features = np.asarray(bond_features, dtype=np.float32)
    connectivity = np.asarray(connectivity)
    bond_transform = np.asarray(bond_transform, dtype=np.float32)
    gru_kernel = np.asarray(gru_kernel, dtype=np.float32)
    gru_rec_kernel = np.asarray(gru_rec_kernel, dtype=np.float32)
    gru_bias = np.asarray(gru_bias, dtype=np.float32)

    # ---- host-side layout prep ----
    # W2[(d,l), m] = bt3[d, l, m]; d-major flatten is chunk-contiguous.
    w2 = (
        bond_transform.reshape(NCHUNK, 128, D)
        .transpose(1, 0, 2)
        .reshape(128, NCHUNK * D)
    )
    # selp: 4 consecutive chunks' selection matrices at partition offsets 0/32/64/96
    # SEL_c[d, p] = 1 where d == 2c + p//64
    selp = np.zeros((128, (NCHUNK // 4) * 128), dtype=np.float32)
    p = np.arange(128)
    for r in range(NCHUNK // 4):
        for j in range(4):
            c = 4 * r + j
            selp[32 * j + 2 * c + p // 64, r * 128 + p] = 1.0
    ident = np.eye(N, dtype=np.float32)
    iotac = np.arange(N, dtype=np.float32).reshape(N, 1)
    iotar4 = np.broadcast_to(np.arange(N, dtype=np.float32), (E, BLK, N)).reshape(
        E, BLK * N
    )
    ones = np.ones((1, 2 * N), dtype=np.float32)
    b0, b1 = gru_bias[0], gru_bias[1]
    biasx = np.concatenate([(b0 + b1)[: 2 * D], b0[2 * D :], b1[2 * D :]]).reshape(
        1, 4 * D
    )

    bfT_all = bond_features.reshape(B * E, BD).T  # [BD, B*E]
    srcf = connectivity[:, :, 0].astype(np.float32)  # [B, E]
    tgtf = connectivity[:, :, 1].astype(np.float32)

    shared = {
        "w2": np.ascontiguousarray(w2).astype(BF16),
        "selp": np.ascontiguousarray(selp).astype(BF16),
        "ident": ident.astype(BF16),
        "iotac": iotac,
        "iotar4": np.ascontiguousarray(iotar4),
        "gruk": np.ascontiguousarray(gru_kernel).astype(BF16),
        "grur": np.ascontiguousarray(gru_rec_kernel).astype(BF16),
        "ones": ones.astype(BF16),
        "biasx": np.ascontiguousarray(biasx).astype(BF16),
    }
    in_maps = []
    for k in range(NCORES):
        bs = slice(k * BPC, (k + 1) * BPC)
        atoms = atom_features[bs]  # [BPC, N, D]
        atomg = (
            np.concatenate([atoms, atoms], axis=2)
            .transpose(1, 0, 2)
            .reshape(N, BPC * 2 * D)
        )
        atomT = atoms.transpose(2, 0, 1).reshape(D, BPC * N)
        hnat = (
            atoms.reshape(NPAIR, 2 * N, D).transpose(1, 0, 2).reshape(2 * N, NPAIR * D)
        )
        bfTk = bfT_all[:, k * BPC * E : (k + 1) * BPC * E]  # [BD, BPC*E]
        bfT4 = np.broadcast_to(bfTk, (4, BD, BPC * E)).reshape(128, BPC * E)
        in_maps.append(
            {
                "atomg": np.ascontiguousarray(atomg).astype(BF16),
                "atomT": np.ascontiguousarray(atomT).astype(BF16),
                "hnat": np.ascontiguousarray(hnat),
                "bfT4": np.ascontiguousarray(bfT4).astype(BF16),
                "srcf": np.ascontiguousarray(srcf[bs].reshape(1, BPC * E)),
                "tgtf": np.ascontiguousarray(tgtf[bs].T),
                **shared,
            }
        )

    nc = _get_nc()
    import os

    kw = {}
    if os.environ.get("KTRACE_DIR"):
        kw["tmpdir"] = os.environ["KTRACE_DIR"]
    res = run_bass_kernel_spmd(nc, in_maps, list(range(NCORES)), **kw)
    _NC_CACHE["last_results"] = res

    parts = []
    for k in range(NCORES):
        o = res.results[k]["out"].reshape(2 * N, NPAIR, D)
        parts.append(o.transpose(1, 0, 2).reshape(BPC, N, D))
    return np.concatenate(parts, axis=0).reshape(B, N, D)
